# revision 1
# baseline (speedup 1.0000x reference)
import sys

if "/opt/trn_rl_repo" not in sys.path:
    sys.path.insert(0, "/opt/trn_rl_repo")

import numpy as np

NCORES = 8
B = 65536
NPC = B // NCORES  # 8192 images per core
G = 8              # image-tiles (of 128) per group
NGROUPS = NPC // (128 * G)
MAGIC = 12582912.0  # 1.5 * 2**23: (v+M)-M == round-to-nearest-even, |v| < 2**22
AF = 128.0 / 127.5

_cache = {}


def _build(wq9, ndve=5):
    """wq9: tuple of 9 floats, quantized conv taps in {0,+-0.5}, row-major.
    ndve: how many of the non-center taps run on DVE (rest on Pool)."""
    from contextlib import ExitStack

    import concourse.tile as tile
    from concourse import bacc, mybir

    f32 = mybir.dt.float32
    f16 = mybir.dt.float16
    Alu = mybir.AluOpType
    Act = mybir.ActivationFunctionType

    nc = bacc.Bacc("TRN2", target_bir_lowering=False, debug=False,
                   num_devices=NCORES)
    x = nc.dram_tensor("x", [NPC, 576], f32, kind="ExternalInput").ap()
    wfc = nc.dram_tensor("wfc", [256, 10], f16, kind="ExternalInput").ap()
    out = nc.dram_tensor("out", [10, NPC], f32, kind="ExternalOutput").ap()

    with tile.TileContext(nc) as tc, ExitStack() as ctx:
        consts = ctx.enter_context(tc.tile_pool(name="consts", bufs=1))
        w1 = consts.tile([128, 10], f16)
        w2 = consts.tile([128, 10], f16)
        nc.sync.dma_start(w1[:], wfc[0:128, :])
        nc.sync.dma_start(w2[:], wfc[128:256, :])

        xpool = ctx.enter_context(tc.tile_pool(name="xp", bufs=2))
        hpool = ctx.enter_context(tc.tile_pool(name="hp", bufs=2))
        yapool = ctx.enter_context(tc.tile_pool(name="yap", bufs=2))
        ybpool = ctx.enter_context(tc.tile_pool(name="ybp", bufs=2))
        ppool = ctx.enter_context(tc.tile_pool(name="pp", bufs=2))
        apool = ctx.enter_context(tc.tile_pool(name="ap", bufs=2))
        tpool = ctx.enter_context(tc.tile_pool(name="tp", bufs=4))
        spool = ctx.enter_context(tc.tile_pool(name="sp", bufs=2))
        po = ctx.enter_context(tc.tile_pool(name="po", bufs=2, space="PSUM"))

        xv_dram = x.rearrange("(g a p) f -> g p a f", p=128, a=G)

        # taps scaled x2 so they land in {0,+-1}: pure add/subtract on A/2
        cen = 2.0 * wq9[4]
        taps = [(dr, dc, 2.0 * wq9[(dr + 1) * 3 + (dc + 1)])
                for dr in (-1, 0, 1) for dc in (-1, 0, 1)
                if not (dr == 0 and dc == 0)
                and wq9[(dr + 1) * 3 + (dc + 1)] != 0.0]
        dve_taps = taps[:ndve]
        pool_taps = taps[ndve:]

        R = G * 24
        for g in range(NGROUPS):
            xt = xpool.tile([128, G * 576], f32)
            nc.sync.dma_start(xt[:].rearrange("p (a f) -> p a f", a=G),
                              xv_dram[g])
            # quantize: A = clamp(round(x*AF - 128), -127, 127); xh = A/2 fp16
            nc.scalar.activation(xt[:], xt[:], Act.Copy,
                                 bias=MAGIC - 128.0, scale=AF)
            nc.vector.tensor_scalar(xt[:], xt[:], MAGIC, -127.0,
                                    Alu.subtract, Alu.max)
            xh = hpool.tile([128, G * 576], f16)
            nc.gpsimd.tensor_scalar(xh[:], xt[:], 127.0, 0.5,
                                    Alu.min, Alu.mult)

            # 3x3 SAME conv (x128 domain) as shifted +-xh adds, split across
            # two accumulators so DVE and Pool run independent chains.
            ya = yapool.tile([128, G * 576], f16)
            yb = ybpool.tile([128, G * 576], f16)
            nc.scalar.activation(ya[:], xh[:], Act.Copy, bias=0.0, scale=cen)
            nc.gpsimd.tensor_scalar_mul(yb[:], xh[:], 0.0)

            xr = xh[:].rearrange("p (r w) -> p r w", w=24)
            xa = xh[:].rearrange("p (a f) -> p a f", a=G)
            for eng, yt, tlist in ((nc.vector, ya, dve_taps),
                                   (nc.gpsimd, yb, pool_taps)):
                yr = yt[:].rearrange("p (r w) -> p r w", w=24)
                yv = yt[:].rearrange("p (a f) -> p a f", a=G)
                for dr, dc, s in tlist:
                    op = Alu.add if s > 0 else Alu.subtract
                    cop = Alu.subtract if s > 0 else Alu.add
                    co0, co1 = max(0, -dc), 24 - max(0, dc)
                    if dr == 0:
                        eng.tensor_tensor(yr[:, :, co0:co1],
                                          yr[:, :, co0:co1],
                                          xr[:, :, co0 + dc:co1 + dc], op)
                        continue
                    r0, r1 = max(0, -dr), R - max(0, dr)
                    eng.tensor_tensor(
                        yr[:, r0:r1, co0:co1], yr[:, r0:r1, co0:co1],
                        xr[:, r0 + dr:r1 + dr, co0 + dc:co1 + dc], op)
                    # cancel cross-image leakage on the G-1 boundary rows
                    if dr == 1:
                        ysl = yv[:, 0:G - 1, 23 * 24 + co0:23 * 24 + co1]
                        xsl = xa[:, 1:G, co0 + dc:co1 + dc]
                    else:
                        ysl = yv[:, 1:G, co0:co1]
                        xsl = xa[:, 0:G - 1,
                                 23 * 24 + co0 + dc:23 * 24 + co1 + dc]
                    eng.tensor_tensor(ysl, ysl, xsl, cop)

            nc.vector.tensor_tensor(ya[:], ya[:], yb[:], Alu.add)

            # maxpool 2x2 -> 12x12 interior (pad ring pools to zero, dropped)
            p1 = ppool.tile([128, G * 288], f16)
            yv4 = ya[:].rearrange("p (r t w) -> p r t w", t=2, w=24)
            p1r = p1[:].rearrange("p (r w) -> p r w", w=24)
            nc.vector.tensor_tensor(p1r, yv4[:, :, 0, :], yv4[:, :, 1, :],
                                    Alu.max)
            act = apool.tile([128, G * 144], f16)
            p1v4 = p1[:].rearrange("p (r w t) -> p r w t", w=12, t=2)
            actr = act[:].rearrange("p (r w) -> p r w", w=12)
            nc.vector.tensor_tensor(actr, p1v4[:, :, :, 0], p1v4[:, :, :, 1],
                                    Alu.max)
            # relu + clip 127 + round (fp16 magic 1536 = 1.5*2**10)
            nc.vector.tensor_scalar(act[:], act[:], 0.0, 127.0,
                                    Alu.max, Alu.min)
            nc.vector.tensor_scalar(act[:], act[:], 1536.0, 1536.0,
                                    Alu.add, Alu.subtract)

            # FC: out^T[o, b] = sum_k W[k, o] actT[k, b], K=144 as two
            # 128-partition matmuls: actT of feats 0:128 vs W_A, and of
            # feats 16:144 vs W_B (zeros except rows 112:128 = feats 128:144)
            for h in range(2):
                aT1 = tpool.tile([128, 512], f16)
                aT2 = tpool.tile([128, 512], f16)
                for j in range(4):
                    a = h * 4 + j
                    nc.sync.dma_start_transpose(
                        aT1[:, j * 128:(j + 1) * 128],
                        act[:, a * 144:a * 144 + 128])
                    nc.sync.dma_start_transpose(
                        aT2[:, j * 128:(j + 1) * 128],
                        act[:, a * 144 + 16:a * 144 + 144])
                pOT = po.tile([10, 512], f32)
                nc.tensor.matmul(pOT[:], w1[:], aT1[:], start=True, stop=False)
                nc.tensor.matmul(pOT[:], w2[:], aT2[:], start=False, stop=True)
                soT = spool.tile([10, 512], f32)
                nc.scalar.copy(soT[:], pOT[:])
                nc.sync.dma_start(
                    out[:, g * 1024 + h * 512:g * 1024 + (h + 1) * 512],
                    soT[:])

    nc.compile()
    return nc


def _prep(conv_w, fc_w):
    # replicate reference weight quantization exactly (all steps exact in f32)
    cw = np.asarray(conv_w, np.float32).reshape(3, 3)
    wq = (np.round(np.clip(cw, -0.5, 0.5) * 2.0) / 2.0).astype(np.float32)
    fw = np.asarray(fc_w, np.float32)
    wfq = (np.round(np.clip(fw, -0.5, 0.5) * 2.0) / 2.0 / 8.0).astype(np.float32)
    # FC sees act128/128; fold the /128 into W (values k/2048, exact fp16).
    # Rows 0:128 = feats 0:128 (W_A); rows 240:256 = feats 128:144 placed at
    # partition 112+ of W_B to match the feats-16:144 transposed tile.
    Wdev = np.zeros((256, 10), np.float32)
    for i in range(12):
        for j in range(12):
            k = i * 12 + j
            r = k if k < 128 else k + 112
            Wdev[r, :] = wfq[:, (i + 1) * 14 + (j + 1)] / 128.0
    return tuple(float(v) for v in wq.flatten()), Wdev.astype(np.float16)


def _get_program(wq9, ndve=5):
    key = (wq9, ndve)
    nc = _cache.get(key)
    if nc is None:
        nc = _build(wq9, ndve)
        _cache[key] = nc
    return nc


def _make_in_maps(x2d, Wdev):
    return [{"x": np.ascontiguousarray(x2d[c * NPC:(c + 1) * NPC]),
             "wfc": Wdev} for c in range(NCORES)]


def run(x, conv_w, fc_w, trace=False, **kw):
    from concourse.bass_utils import run_bass_kernel_spmd

    x2d = np.ascontiguousarray(
        np.asarray(x, np.float32).reshape(B, 576))
    wq9, Wdev = _prep(conv_w, fc_w)
    nc = _get_program(wq9)
    res = run_bass_kernel_spmd(nc, _make_in_maps(x2d, Wdev),
                               core_ids=list(range(NCORES)),
                               trace=trace, **kw)
    out = np.concatenate([np.asarray(r["out"]).T for r in res.results], axis=0)
    return np.ascontiguousarray(out.astype(np.float32)), res


def kernel(x, conv_w, fc_w):
    out, _ = run(x, conv_w, fc_w, trace=False)
    return out



# revision 2
# speedup vs baseline: 3.0336x; 3.0336x over previous
import sys

if "/opt/trn_rl_repo" not in sys.path:
    sys.path.insert(0, "/opt/trn_rl_repo")

import numpy as np

NCORES = 8
B = 65536
NPC = B // NCORES  # 8192 images per core
G = 8              # image-tiles (of 128) per group
NGROUPS = NPC // (128 * G)
AF = 128.0 / 127.5
OFF = 1536.0  # f16 magic: adding 1536 rounds |v|<512 to integer (RNE)

_cache = {}


def _build(sgn9, np_taps=2):
    """sgn9: tuple of 9 ints in {-1,0,1}, conv taps (2x quantized), row-major.
    np_taps: how many taps run on the GpSimd(Pool) chain (rest on DVE)."""
    from contextlib import ExitStack

    import concourse.tile as tile
    import concourse.masks as masks
    from concourse import bacc, mybir

    f32 = mybir.dt.float32
    f16 = mybir.dt.float16
    Alu = mybir.AluOpType
    Act = mybir.ActivationFunctionType

    nc = bacc.Bacc("TRN2", target_bir_lowering=False, debug=False,
                   num_devices=NCORES)
    x = nc.dram_tensor("x", [NPC, 576], f32, kind="ExternalInput").ap()
    w1 = nc.dram_tensor("w1", [128, 10], f16, kind="ExternalInput").ap()
    w2 = nc.dram_tensor("w2", [16, 10], f16, kind="ExternalInput").ap()
    out = nc.dram_tensor("out", [10, NPC], f32, kind="ExternalOutput").ap()

    cen = float(sgn9[4])
    taps = [(dr, dc, float(sgn9[(dr + 1) * 3 + (dc + 1)]))
            for dr in (-1, 0, 1) for dc in (-1, 0, 1)
            if not (dr == 0 and dc == 0)
            and sgn9[(dr + 1) * 3 + (dc + 1)] != 0]
    # GpSimd chain prefers dr==0 taps (no image-boundary cancels needed)
    taps.sort(key=lambda t: (t[0] != 0, t[0], t[1]))
    np_taps = min(np_taps, len(taps))
    p_taps = taps[:np_taps]
    v_taps = taps[np_taps:]

    R = G * 24

    with tile.TileContext(nc) as tc, ExitStack() as ctx:
        consts = ctx.enter_context(tc.tile_pool(name="consts", bufs=1))
        w1t = consts.tile([128, 10], f16)
        w2t = consts.tile([16, 10], f16)
        ident = consts.tile([128, 128], f16)
        nc.sync.dma_start(w1t[:], w1)
        nc.sync.dma_start(w2t[:], w2)
        masks.make_identity(nc, ident[:])

        xpool = ctx.enter_context(tc.tile_pool(name="xp", bufs=2))
        hpool = ctx.enter_context(tc.tile_pool(name="hp", bufs=2))
        yapool = ctx.enter_context(tc.tile_pool(name="yap", bufs=2))
        ybpool = ctx.enter_context(tc.tile_pool(name="ybp", bufs=2))
        ppool = ctx.enter_context(tc.tile_pool(name="pp", bufs=2))
        apool = ctx.enter_context(tc.tile_pool(name="ap", bufs=2))
        tpool = ctx.enter_context(tc.tile_pool(name="tp", bufs=4))
        spool = ctx.enter_context(tc.tile_pool(name="sp", bufs=2))
        pst = ctx.enter_context(tc.tile_pool(name="pst", bufs=2, space="PSUM"))
        ps2 = ctx.enter_context(tc.tile_pool(name="ps2", bufs=2, space="PSUM"))
        po = ctx.enter_context(tc.tile_pool(name="po", bufs=2, space="PSUM"))

        xv_dram = x.rearrange("(g a p) f -> g p a f", p=128, a=G)

        def emit_tap(eng, yt, dr, dc, s, first=False):
            """Accumulate s * shift(xh, dr, dc) into yt (or init yt if first).
            Returns ops via closure vars xr/xa/yt views."""
            op = Alu.add if s > 0 else Alu.subtract
            cop = Alu.subtract if s > 0 else Alu.add
            co0, co1 = max(0, -dc), 24 - max(0, dc)
            yr = yt[:].rearrange("p (r w) -> p r w", w=24)
            yv = yt[:].rearrange("p (a f) -> p a f", a=G)
            if dr == 0:
                if first:
                    assert s > 0
                    # zero the excluded column, then copy the shifted data
                    if co0 > 0:
                        nc.gpsimd.memset(yr[:, :, 0:co0], 0.0)
                    if co1 < 24:
                        nc.gpsimd.memset(yr[:, :, co1:24], 0.0)
                    nc.gpsimd.tensor_copy(yr[:, :, co0:co1],
                                          xr[:, :, co0 + dc:co1 + dc])
                else:
                    eng.tensor_tensor(yr[:, :, co0:co1], yr[:, :, co0:co1],
                                      xr[:, :, co0 + dc:co1 + dc], op)
                return
            r0, r1 = max(0, -dr), R - max(0, dr)
            if first:
                assert s > 0
                nc.gpsimd.memset(yt[:], 0.0)
                nc.gpsimd.tensor_copy(yr[:, r0:r1, co0:co1],
                                      xr[:, r0 + dr:r1 + dr, co0 + dc:co1 + dc])
            else:
                eng.tensor_tensor(
                    yr[:, r0:r1, co0:co1], yr[:, r0:r1, co0:co1],
                    xr[:, r0 + dr:r1 + dr, co0 + dc:co1 + dc], op)
            # cancel cross-image leakage on the G-1 boundary rows
            if dr == 1:
                ysl = yv[:, 0:G - 1, 23 * 24 + co0:23 * 24 + co1]
                xsl = xa[:, 1:G, co0 + dc:co1 + dc]
            else:
                ysl = yv[:, 1:G, co0:co1]
                xsl = xa[:, 0:G - 1, 23 * 24 + co0 + dc:23 * 24 + co1 + dc]
            eng.tensor_tensor(ysl, ysl, xsl, cop)

        for g in range(NGROUPS):
            xt = xpool.tile([128, G * 576], f32)
            nc.sync.dma_start(xt[:].rearrange("p (a f) -> p a f", a=G),
                              xv_dram[g])
            # quantize: xh = clamp(round(x*AF - 128), -127, .) as A (int, f16)
            # One scalar activation: f16 write of x*AF + (OFF-128) rounds to
            # integer (RNE) since values land in [1408, 1664] where f16 ulp=1.
            xh = hpool.tile([128, G * 576], f16)
            nc.scalar.activation(xh[:], xt[:], Act.Copy,
                                 bias=OFF - 128.0, scale=AF)
            # subtract magic offset + low clamp (DVE 4x mode)
            nc.vector.tensor_scalar(xh[:], xh[:], OFF, -127.0,
                                    Alu.subtract, Alu.max)

            xr = xh[:].rearrange("p (r w) -> p r w", w=24)
            xa = xh[:].rearrange("p (a f) -> p a f", a=G)

            # DVE chain: ya = cen*xh + sum of v_taps
            ya = yapool.tile([128, G * 576], f16)
            nc.scalar.activation(ya[:], xh[:], Act.Copy, bias=0.0, scale=cen)
            for dr, dc, s in v_taps:
                emit_tap(nc.vector, ya, dr, dc, s)

            # GpSimd chain: yb = sum of p_taps (sign-normalized so the first
            # tap is a plain copy); fold chain sign into the combine op.
            if p_taps:
                chs = p_taps[0][2]
                yb = ybpool.tile([128, G * 576], f16)
                for i, (dr, dc, s) in enumerate(p_taps):
                    emit_tap(nc.gpsimd, yb, dr, dc, s * chs, first=(i == 0))
                nc.vector.tensor_tensor(ya[:], ya[:], yb[:],
                                        Alu.add if chs > 0 else Alu.subtract)

            # maxpool 2x2 over the 24x24 conv output -> 12x12
            p1 = ppool.tile([128, G * 288], f16)
            yv4 = ya[:].rearrange("p (r t w) -> p r t w", t=2, w=24)
            p1r = p1[:].rearrange("p (r w) -> p r w", w=24)
            nc.vector.tensor_tensor(p1r, yv4[:, :, 0, :], yv4[:, :, 1, :],
                                    Alu.max)
            act = apool.tile([128, G * 144], f16)
            p1v = p1[:].rearrange("p (r w t) -> p r w t", w=12, t=2)
            actr = act[:].rearrange("p (r w) -> p r w", w=12)
            nc.vector.tensor_tensor(actr, p1v[:, :, :, 0], p1v[:, :, :, 1],
                                    Alu.max)
            # act = round(clamp(y/2, 0, 127)); ya is 2*y_ref*128 (integer)
            nc.vector.tensor_scalar(act[:], act[:], 0.5, 0.0,
                                    Alu.mult, Alu.max)
            nc.vector.tensor_scalar(act[:], act[:], 127.0, OFF,
                                    Alu.min, Alu.add)
            nc.vector.tensor_scalar(act[:], act[:], OFF, None,
                                    Alu.subtract, Alu.bypass)

            # FC: transpose act via the PE (identity matmul) into PSUM,
            # copy to SBUF, then out^T[o, b] = W^T @ actT
            for h in range(2):
                pT1 = pst.tile([128, 512], f16)
                pT2 = ps2.tile([128, 512], f16)
                for j in range(4):
                    a = h * 4 + j
                    nc.tensor.transpose(pT1[:, j * 128:(j + 1) * 128],
                                        act[:, a * 144:a * 144 + 128],
                                        ident[:])
                    nc.tensor.transpose(pT2[0:16, j * 128:(j + 1) * 128],
                                        act[:, a * 144 + 128:(a + 1) * 144],
                                        ident[:])
                aT1 = tpool.tile([128, 512], f16)
                aT2 = tpool.tile([16, 512], f16)
                nc.scalar.copy(aT1[:], pT1[:])
                nc.scalar.copy(aT2[:], pT2[0:16, :])
                pOT = po.tile([10, 512], f32)
                nc.tensor.matmul(pOT[:], w1t[:], aT1[:], start=True,
                                 stop=False)
                nc.tensor.matmul(pOT[:], w2t[:], aT2[:], start=False,
                                 stop=True)
                soT = spool.tile([10, 512], f32)
                nc.scalar.copy(soT[:], pOT[:])
                nc.sync.dma_start(
                    out[:, g * 1024 + h * 512:g * 1024 + (h + 1) * 512],
                    soT[:])

    nc.compile()
    return nc


def _prep(conv_w, fc_w):
    # replicate reference weight quantization exactly (all steps exact in f32)
    cw = np.asarray(conv_w, np.float32).reshape(3, 3)
    sgn9 = tuple(int(v) for v in
                 np.round(np.clip(cw, -0.5, 0.5) * 2.0).flatten())
    fw = np.asarray(fc_w, np.float32)
    wfq = (np.round(np.clip(fw, -0.5, 0.5) * 2.0) / 2.0 / 8.0).astype(
        np.float32)
    # FC sees act/128; fold the /128 into W (values m/2048, exact fp16).
    # Feature k = i*12 + j of the 12x12 pool interior maps to fc column
    # (i+1)*14 + (j+1) of the 14x14 padded pool grid.
    Wdev = np.zeros((144, 10), np.float32)
    for i in range(12):
        for j in range(12):
            Wdev[i * 12 + j, :] = wfq[:, (i + 1) * 14 + (j + 1)] / 128.0
    W1 = np.ascontiguousarray(Wdev[0:128]).astype(np.float16)
    W2 = np.ascontiguousarray(Wdev[128:144]).astype(np.float16)
    return sgn9, W1, W2


def _get_program(sgn9, np_taps=2):
    key = (sgn9, np_taps)
    nc = _cache.get(key)
    if nc is None:
        nc = _build(sgn9, np_taps)
        _cache[key] = nc
    return nc


def _make_in_maps(x2d, W1, W2):
    return [{"x": np.ascontiguousarray(x2d[c * NPC:(c + 1) * NPC]),
             "w1": W1, "w2": W2} for c in range(NCORES)]


def run(x, conv_w, fc_w, trace=False, np_taps=2, **kw):
    from concourse.bass_utils import run_bass_kernel_spmd

    x2d = np.ascontiguousarray(
        np.asarray(x, np.float32).reshape(B, 576))
    sgn9, W1, W2 = _prep(conv_w, fc_w)
    nc = _get_program(sgn9, np_taps)
    res = run_bass_kernel_spmd(nc, _make_in_maps(x2d, W1, W2),
                               core_ids=list(range(NCORES)),
                               trace=trace, **kw)
    out = np.concatenate([np.asarray(r["out"]).T for r in res.results],
                         axis=0)
    return np.ascontiguousarray(out.astype(np.float32)), res


def kernel(x, conv_w, fc_w):
    out, _ = run(x, conv_w, fc_w, trace=False)
    return out


# revision 10
# speedup vs baseline: 4.5978x; 1.5156x over previous
import sys

if "/opt/trn_rl_repo" not in sys.path:
    sys.path.insert(0, "/opt/trn_rl_repo")

import numpy as np

NCORES = 8
B = 65536
NPC = B // NCORES  # 8192 images per core
G = 8              # image-tiles (of 128) per group
NGROUPS = NPC // (128 * G)
AF = 128.0 / 127.5
OFF = 1536.0   # f16 magic: +1536 makes the f16 write round |v|<512 to int
PAD = 127.0    # padded-cell value == quantized level of A=0, post-clamp space
# padded image slot: lead spacer row of 26, then per image 25 rows x 26 cols
# (24x24 data at rows 0..23, cols 1..24; row 24 is the inter-image spacer)
SLOT = 650     # 25 * 26
L = 26 + G * SLOT  # 5226 padded elements per partition

_cache = {}


def _build(sgn9):
    """sgn9: tuple of 9 ints in {-1,0,1}: conv taps (2x quantized), row-major.

    Layout: partition = image, free = padded pixels. All 3x3 taps become
    single contiguous shifted adds over the full tile; padded cells hold
    127 (== A=0 in the offset space xh = clamp(A,-127,127)+127) so every
    data pixel receives every tap and the uniform offset 127*sum(signs)
    is folded into the activation-quant bias. Partial sums stay < 2048 in
    magnitude for <= 8 accumulation terms, so f16 accumulation is exact.
    """
    from contextlib import ExitStack

    import concourse.tile as tile
    import concourse.masks as masks
    from concourse import bacc, mybir

    f32 = mybir.dt.float32
    f16 = mybir.dt.float16
    Alu = mybir.AluOpType
    Act = mybir.ActivationFunctionType

    nc = bacc.Bacc("TRN2", target_bir_lowering=False, debug=False,
                   num_devices=NCORES)
    x = nc.dram_tensor("x", [NPC, 576], f32, kind="ExternalInput").ap()
    w1 = nc.dram_tensor("w1", [128, 10], f16, kind="ExternalInput").ap()
    w2 = nc.dram_tensor("w2", [16, 10], f16, kind="ExternalInput").ap()
    out = nc.dram_tensor("out", [10, NPC], f32, kind="ExternalOutput").ap()

    cen = float(sgn9[4])
    taps = [(26 * dr + dc, float(sgn9[(dr + 1) * 3 + (dc + 1)]))
            for dr in (-1, 0, 1) for dc in (-1, 0, 1)
            if not (dr == 0 and dc == 0)
            and sgn9[(dr + 1) * 3 + (dc + 1)] != 0]
    ksum = cen + sum(s for _, s in taps)  # offset multiplier (uniform)
    # GpSimd initializes ya with a plain shifted copy of a positive tap
    # (only possible when there is no center contribution)
    ci = next((i for i, (_, s) in enumerate(taps) if s > 0), None)
    if cen == 0.0 and ci is not None:
        copy_tap = taps.pop(ci)
    else:
        copy_tap = None
    # GpSimd also takes one accumulate tap (tt add/sub is supported there)
    p_taps, v_taps = (taps[:1], taps[1:]) if len(taps) > 4 else ([], taps)

    with tile.TileContext(nc) as tc, ExitStack() as ctx:
        consts = ctx.enter_context(tc.tile_pool(name="consts", bufs=1))
        w1t = consts.tile([128, 10], f16)
        w2t = consts.tile([16, 10], f16)
        ident = consts.tile([128, 128], f16)
        nc.sync.dma_start(w1t[:], w1)
        nc.sync.dma_start(w2t[:], w2)
        masks.make_identity(nc, ident[:])
        # per-partition f32 bias constants for non-Copy activations
        bias_vals = [OFF + 127.0, 254.0, 127.0 + 63.5 * ksum, 127.0,
                     OFF, -OFF]
        biases = {}
        for i, v in enumerate(bias_vals):
            bt = consts.tile([128, 1], f32, name=f"bias{i}")
            nc.gpsimd.memset(bt[:], v)
            biases[v] = bt

        xpool = ctx.enter_context(tc.tile_pool(name="xp", bufs=2))
        hpool = ctx.enter_context(tc.tile_pool(name="hp", bufs=1))
        spool_s = ctx.enter_context(tc.tile_pool(name="xs", bufs=2))
        yapool = ctx.enter_context(tc.tile_pool(name="yap", bufs=2))
        ppool = ctx.enter_context(tc.tile_pool(name="pp", bufs=2))
        apool = ctx.enter_context(tc.tile_pool(name="ap", bufs=2))
        tpool = ctx.enter_context(tc.tile_pool(name="tp", bufs=4))
        spool = ctx.enter_context(tc.tile_pool(name="sp", bufs=2))
        pst = ctx.enter_context(tc.tile_pool(name="pst", bufs=2, space="PSUM"))
        ps2 = ctx.enter_context(tc.tile_pool(name="ps2", bufs=2, space="PSUM"))
        po = ctx.enter_context(tc.tile_pool(name="po", bufs=2, space="PSUM"))

        # manual double-buffer for xh: padded cells are initialized once and
        # never rewritten (quant + clamps touch only data cells)
        xhs = [hpool.tile([128, L], f16, name=f"xh{i}") for i in range(2)]
        for xh in xhs:
            xv = xh[:, 26:].rearrange("p (a r c) -> p a r c", r=25, c=26)
            nc.gpsimd.memset(xh[:, 0:26], PAD)         # lead spacer row
            nc.gpsimd.memset(xv[:, :, 24, :], PAD)     # inter-image spacers
            nc.gpsimd.memset(xv[:, :, 0:24, 0:1], PAD)   # left border col
            nc.gpsimd.memset(xv[:, :, 0:24, 25:26], PAD)  # right border col

        xv_dram = x.rearrange("(g a p) f -> g p a f", p=128, a=G)

        for g in range(NGROUPS):
            xt = xpool.tile([128, G * 576], f32)
            nc.sync.dma_start(xt[:].rearrange("p (a f) -> p a f", a=G),
                              xv_dram[g])
            xh = xhs[g % 2]
            # data-cell view of the padded tile: [a, r(24), c(24)] at col 1
            xv = xh[:, 26:].rearrange("p (a r c) -> p a r c", r=25, c=26)
            xdv = xv[:, :, 0:24, 1:25]
            # quant: xh' = A + 1536 via f16 magic write (A = round(x*AF-128))
            nc.scalar.activation(
                xdv, xt[:].rearrange("p (a r c) -> p a r c", r=24, c=24),
                Act.Copy, bias=OFF - 128.0, scale=AF)
            # both clamps via two Relus (scalar engine), leaving
            # xh = clamp(A, -127, 127) + 127 in [0, 254]
            xs = spool_s.tile([128, G * 576], f16)
            xsv = xs[:].rearrange("p (a r c) -> p a r c", r=24, c=24)
            nc.scalar.activation(xsv, xdv, Act.Relu,
                                 bias=biases[OFF + 127.0][:], scale=-1.0)
            nc.scalar.activation(xdv, xsv, Act.Relu,
                                 bias=biases[254.0][:], scale=-1.0)

            # conv: ya = sum_s sign * shift(xh, s) (+127*ksum offset)
            ya = yapool.tile([128, L], f16)
            yv = ya[:, 26:].rearrange("p (a r c) -> p a r c", r=25, c=26)
            if copy_tap is not None:
                s0, _ = copy_tap
                b0, b1 = max(0, -s0), L - max(0, s0)
                if b0 > 0:
                    nc.gpsimd.memset(ya[:, 0:b0], 0.0)
                if b1 < L:
                    nc.gpsimd.memset(ya[:, b1:L], 0.0)
                nc.gpsimd.tensor_copy(ya[:, b0:b1], xh[:, b0 + s0:b1 + s0])
            elif cen != 0.0:
                nc.scalar.activation(yv[:, :, 0:24, 1:25], xdv, Act.Copy,
                                     bias=0.0, scale=cen)
            else:
                nc.gpsimd.memset(ya[:], 0.0)
            for off, s in p_taps:
                b0, b1 = max(0, -off), L - max(0, off)
                nc.gpsimd.tensor_tensor(
                    ya[:, b0:b1], ya[:, b0:b1], xh[:, b0 + off:b1 + off],
                    Alu.add if s > 0 else Alu.subtract)
            for off, s in v_taps:
                b0, b1 = max(0, -off), L - max(0, off)
                nc.vector.tensor_tensor(
                    ya[:, b0:b1], ya[:, b0:b1], xh[:, b0 + off:b1 + off],
                    Alu.add if s > 0 else Alu.subtract)

            # maxpool 2x2 over the 24x24 conv output -> 12x12 (DVE).
            # Column pairs first (stride-2 reads, 1x mode), then row pairs
            # on the half-sized tensor (packed reads, 2x mode).
            yu = yv[:, :, 0:24, 1:25].rearrange("p a r (c u) -> p a r c u",
                                                u=2)
            p1 = ppool.tile([128, G * 288], f16)
            p1v = p1[:].rearrange("p (a r c) -> p a r c", r=24, c=12)
            nc.vector.tensor_tensor(p1v, yu[:, :, :, :, 0],
                                    yu[:, :, :, :, 1], Alu.max)
            act = apool.tile([128, G * 144], f16)
            p1w = p1[:].rearrange("p (a r t c) -> p a r t c",
                                  r=12, t=2, c=12)
            actr = act[:].rearrange("p (a r c) -> p a r c", r=12, c=12)
            nc.vector.tensor_tensor(actr, p1w[:, :, :, 0, :],
                                    p1w[:, :, :, 1, :], Alu.max)
            # act-quant on the scalar engine:
            #   u = p2 - 127*ksum;  act = round(clamp(u/2, 0, 127))
            # t1 = relu(127 + 63.5*ksum - p2/2) = 127 - min(u/2, 127)
            # t2 = relu(127 - t1)               = clamp(u/2, 0, 127)
            # t3 = t2 + 1536 (f16 write rounds), t4 = t3 - 1536
            nc.scalar.activation(act[:], act[:], Act.Relu,
                                 bias=biases[127.0 + 63.5 * ksum][:],
                                 scale=-0.5)
            nc.scalar.activation(act[:], act[:], Act.Relu,
                                 bias=biases[127.0][:], scale=-1.0)
            nc.scalar.activation(act[:], act[:], Act.Identity,
                                 bias=biases[OFF][:], scale=1.0)
            nc.scalar.activation(act[:], act[:], Act.Identity,
                                 bias=biases[-OFF][:], scale=1.0)

            # FC: transpose act via the PE (identity matmul) into PSUM,
            # copy to SBUF, then out^T[o, b] = W^T @ actT
            for h in range(2):
                pT1 = pst.tile([128, 512], f16)
                pT2 = ps2.tile([128, 512], f16)
                for j in range(4):
                    a = h * 4 + j
                    nc.tensor.transpose(pT1[:, j * 128:(j + 1) * 128],
                                        act[:, a * 144:a * 144 + 128],
                                        ident[:])
                    nc.tensor.transpose(pT2[0:16, j * 128:(j + 1) * 128],
                                        act[:, a * 144 + 128:(a + 1) * 144],
                                        ident[:])
                aT1 = tpool.tile([128, 512], f16)
                aT2 = tpool.tile([16, 512], f16)
                nc.scalar.copy(aT1[:], pT1[:])
                nc.scalar.copy(aT2[:], pT2[0:16, :])
                pOT = po.tile([10, 512], f32)
                nc.tensor.matmul(pOT[:], w1t[:], aT1[:], start=True,
                                 stop=False)
                nc.tensor.matmul(pOT[:], w2t[:], aT2[:], start=False,
                                 stop=True)
                soT = spool.tile([10, 512], f32)
                nc.scalar.copy(soT[:], pOT[:])
                nc.sync.dma_start(
                    out[:, g * 1024 + h * 512:g * 1024 + (h + 1) * 512],
                    soT[:])

    nc.compile()
    return nc


def _prep(conv_w, fc_w):
    # replicate reference weight quantization exactly (all steps exact in f32)
    cw = np.asarray(conv_w, np.float32).reshape(3, 3)
    sgn9 = tuple(int(v) for v in
                 np.round(np.clip(cw, -0.5, 0.5) * 2.0).flatten())
    fw = np.asarray(fc_w, np.float32)
    wfq = (np.round(np.clip(fw, -0.5, 0.5) * 2.0) / 2.0 / 8.0).astype(
        np.float32)
    # FC sees act/128; fold the /128 into W (values m/2048, exact fp16).
    # Feature k = i*12 + j of the 12x12 pool interior maps to fc column
    # (i+1)*14 + (j+1) of the 14x14 padded pool grid.
    Wdev = np.zeros((144, 10), np.float32)
    for i in range(12):
        for j in range(12):
            Wdev[i * 12 + j, :] = wfq[:, (i + 1) * 14 + (j + 1)] / 128.0
    W1 = np.ascontiguousarray(Wdev[0:128]).astype(np.float16)
    W2 = np.ascontiguousarray(Wdev[128:144]).astype(np.float16)
    return sgn9, W1, W2


def _get_program(sgn9):
    nc = _cache.get(sgn9)
    if nc is None:
        nc = _build(sgn9)
        _cache[sgn9] = nc
    return nc


def _make_in_maps(x2d, W1, W2):
    return [{"x": np.ascontiguousarray(x2d[c * NPC:(c + 1) * NPC]),
             "w1": W1, "w2": W2} for c in range(NCORES)]


def run(x, conv_w, fc_w, trace=False, **kw):
    from concourse.bass_utils import run_bass_kernel_spmd

    x2d = np.ascontiguousarray(
        np.asarray(x, np.float32).reshape(B, 576))
    sgn9, W1, W2 = _prep(conv_w, fc_w)
    nc = _get_program(sgn9)
    res = run_bass_kernel_spmd(nc, _make_in_maps(x2d, W1, W2),
                               core_ids=list(range(NCORES)),
                               trace=trace, **kw)
    out = np.concatenate([np.asarray(r["out"]).T for r in res.results],
                         axis=0)
    return np.ascontiguousarray(out.astype(np.float32)), res


def kernel(x, conv_w, fc_w):
    out, _ = run(x, conv_w, fc_w, trace=False)
    return out


# revision 13
# speedup vs baseline: 4.7363x; 1.0301x over previous
import sys

if "/opt/trn_rl_repo" not in sys.path:
    sys.path.insert(0, "/opt/trn_rl_repo")

import numpy as np

NCORES = 8
B = 65536
NPC = B // NCORES  # 8192 images per core
G = 8              # image-tiles (of 128) per group
NGROUPS = NPC // (128 * G)
AF = 128.0 / 127.5
OFF = 1536.0   # f16 magic: +1536 makes the f16 write round |v|<512 to int
PAD = 127.0    # padded-cell value == quantized level of A=0, post-clamp space
# padded image slot: lead spacer row of 26, then per image 25 rows x 26 cols
# (24x24 data at rows 0..23, cols 1..24; row 24 is the inter-image spacer)
SLOT = 650     # 25 * 26
L = 26 + G * SLOT  # 5226 padded elements per partition

_cache = {}


def _build(sgn9):
    """sgn9: tuple of 9 ints in {-1,0,1}: conv taps (2x quantized), row-major.

    Layout: partition = image, free = padded pixels. All 3x3 taps become
    single contiguous shifted adds over the full tile; padded cells hold
    127 (== A=0 in the offset space xh = clamp(A,-127,127)+127) so every
    data pixel receives every tap and the uniform offset 127*sum(signs)
    is folded into the activation-quant bias. Partial sums stay < 2048 in
    magnitude for <= 8 accumulation terms, so f16 accumulation is exact.
    """
    from contextlib import ExitStack

    import concourse.tile as tile
    import concourse.masks as masks
    from concourse import bacc, mybir

    f32 = mybir.dt.float32
    f16 = mybir.dt.float16
    Alu = mybir.AluOpType
    Act = mybir.ActivationFunctionType

    nc = bacc.Bacc("TRN2", target_bir_lowering=False, debug=False,
                   num_devices=NCORES)
    x = nc.dram_tensor("x", [NPC, 576], f32, kind="ExternalInput").ap()
    w1 = nc.dram_tensor("w1", [128, 10], f16, kind="ExternalInput").ap()
    w2 = nc.dram_tensor("w2", [16, 10], f16, kind="ExternalInput").ap()
    out = nc.dram_tensor("out", [10, NPC], f32, kind="ExternalOutput").ap()

    cen = float(sgn9[4])
    taps = [(26 * dr + dc, float(sgn9[(dr + 1) * 3 + (dc + 1)]))
            for dr in (-1, 0, 1) for dc in (-1, 0, 1)
            if not (dr == 0 and dc == 0)
            and sgn9[(dr + 1) * 3 + (dc + 1)] != 0]
    ksum = cen + sum(s for _, s in taps)  # offset multiplier (uniform)
    # GpSimd initializes ya with a plain shifted copy of a positive tap
    # (only possible when there is no center contribution)
    ci = next((i for i, (_, s) in enumerate(taps) if s > 0), None)
    if cen == 0.0 and ci is not None:
        copy_tap = taps.pop(ci)
    else:
        copy_tap = None
    # GpSimd also takes one accumulate tap (tt add/sub is supported there)
    p_taps, v_taps = (taps[:1], taps[1:]) if len(taps) > 4 else ([], taps)

    with tile.TileContext(nc) as tc, ExitStack() as ctx:
        consts = ctx.enter_context(tc.tile_pool(name="consts", bufs=1))
        w1t = consts.tile([128, 10], f16)
        w2t = consts.tile([16, 10], f16)
        ident = consts.tile([128, 128], f16)
        nc.sync.dma_start(w1t[:], w1)
        nc.sync.dma_start(w2t[:], w2)
        masks.make_identity(nc, ident[:])
        # per-partition f32 bias constants for non-Copy activations
        bias_vals = [OFF + 127.0, 254.0, 127.0 + 63.5 * ksum, 127.0,
                     OFF, -OFF]
        biases = {}
        for i, v in enumerate(bias_vals):
            bt = consts.tile([128, 1], f32, name=f"bias{i}")
            nc.gpsimd.memset(bt[:], v)
            biases[v] = bt

        xpool = ctx.enter_context(tc.tile_pool(name="xp", bufs=2))
        hpool = ctx.enter_context(tc.tile_pool(name="hp", bufs=1))
        spool_s = ctx.enter_context(tc.tile_pool(name="xs", bufs=2))
        yapool = ctx.enter_context(tc.tile_pool(name="yap", bufs=2))
        ppool = ctx.enter_context(tc.tile_pool(name="pp", bufs=2))
        apool = ctx.enter_context(tc.tile_pool(name="ap", bufs=2))
        tpool = ctx.enter_context(tc.tile_pool(name="tp", bufs=4))
        spool = ctx.enter_context(tc.tile_pool(name="sp", bufs=2))
        pst = ctx.enter_context(tc.tile_pool(name="pst", bufs=2, space="PSUM"))
        po = ctx.enter_context(tc.tile_pool(name="po", bufs=2, space="PSUM"))

        # manual double-buffer for xh: padded cells are initialized once and
        # never rewritten (quant + clamps touch only data cells)
        xhs = [hpool.tile([128, L], f16, name=f"xh{i}") for i in range(2)]
        for xh in xhs:
            xv = xh[:, 26:].rearrange("p (a r c) -> p a r c", r=25, c=26)
            nc.gpsimd.memset(xh[:, 0:26], PAD)         # lead spacer row
            nc.gpsimd.memset(xv[:, :, 24, :], PAD)     # inter-image spacers
            nc.gpsimd.memset(xv[:, :, 0:24, 0:1], PAD)   # left border col
            nc.gpsimd.memset(xv[:, :, 0:24, 25:26], PAD)  # right border col

        xv_dram = x.rearrange("(g a p) f -> g p a f", p=128, a=G)

        def stage_a(g):
            """DMA in, quantize + clamp (Scalar), start conv (GpSimd)."""
            xt = xpool.tile([128, G * 576], f32, name="xt")
            nc.sync.dma_start(xt[:].rearrange("p (a f) -> p a f", a=G),
                              xv_dram[g])
            xh = xhs[g % 2]
            xv = xh[:, 26:].rearrange("p (a r c) -> p a r c", r=25, c=26)
            xdv = xv[:, :, 0:24, 1:25]
            # quant: xh' = A + 1536 via f16 magic write (A = round(x*AF-128))
            nc.scalar.activation(
                xdv, xt[:].rearrange("p (a r c) -> p a r c", r=24, c=24),
                Act.Copy, bias=OFF - 128.0, scale=AF)
            # both clamps via two Relus (scalar engine), leaving
            # xh = clamp(A, -127, 127) + 127 in [0, 254]
            xs = spool_s.tile([128, G * 576], f16, name="xs")
            xsv = xs[:].rearrange("p (a r c) -> p a r c", r=24, c=24)
            nc.scalar.activation(xsv, xdv, Act.Relu,
                                 bias=biases[OFF + 127.0][:], scale=-1.0)
            nc.scalar.activation(xdv, xsv, Act.Relu,
                                 bias=biases[254.0][:], scale=-1.0)

            # GpSimd part of the conv: init ya + its accumulate taps
            ya = yapool.tile([128, L], f16, name="ya")
            yv = ya[:, 26:].rearrange("p (a r c) -> p a r c", r=25, c=26)
            if copy_tap is not None:
                s0, _ = copy_tap
                b0, b1 = max(0, -s0), L - max(0, s0)
                if b0 > 0:
                    nc.gpsimd.memset(ya[:, 0:b0], 0.0)
                if b1 < L:
                    nc.gpsimd.memset(ya[:, b1:L], 0.0)
                nc.gpsimd.tensor_copy(ya[:, b0:b1], xh[:, b0 + s0:b1 + s0])
            elif cen != 0.0:
                nc.scalar.activation(yv[:, :, 0:24, 1:25], xdv, Act.Copy,
                                     bias=0.0, scale=cen)
            else:
                nc.gpsimd.memset(ya[:], 0.0)
            for off, s in p_taps:
                b0, b1 = max(0, -off), L - max(0, off)
                nc.gpsimd.tensor_tensor(
                    ya[:, b0:b1], ya[:, b0:b1], xh[:, b0 + off:b1 + off],
                    Alu.add if s > 0 else Alu.subtract)
            return xh, ya

        def stage_b(g, st):
            """DVE taps + maxpool."""
            xh, ya = st
            yv = ya[:, 26:].rearrange("p (a r c) -> p a r c", r=25, c=26)
            for off, s in v_taps:
                b0, b1 = max(0, -off), L - max(0, off)
                nc.vector.tensor_tensor(
                    ya[:, b0:b1], ya[:, b0:b1], xh[:, b0 + off:b1 + off],
                    Alu.add if s > 0 else Alu.subtract)
            # maxpool 2x2: column pairs first (stride-2 reads, 1x mode),
            # then row pairs on the half-sized tensor (packed, 2x mode)
            yu = yv[:, :, 0:24, 1:25].rearrange("p a r (c u) -> p a r c u",
                                                u=2)
            p1 = ppool.tile([128, G * 288], f16, name="p1")
            p1v = p1[:].rearrange("p (a r c) -> p a r c", r=24, c=12)
            nc.vector.tensor_tensor(p1v, yu[:, :, :, :, 0],
                                    yu[:, :, :, :, 1], Alu.max)
            act = apool.tile([128, G * 144], f16, name="act")
            p1w = p1[:].rearrange("p (a r t c) -> p a r t c",
                                  r=12, t=2, c=12)
            actr = act[:].rearrange("p (a r c) -> p a r c", r=12, c=12)
            nc.vector.tensor_tensor(actr, p1w[:, :, :, 0, :],
                                    p1w[:, :, :, 1, :], Alu.max)
            return act

        def stage_c(g, act):
            """Act-quant (Scalar), PE transpose + FC, output DMA."""
            #   u = p2 - 127*ksum;  act = round(clamp(u/2, 0, 127))
            # t1 = relu(127 + 63.5*ksum - p2/2) = 127 - min(u/2, 127)
            # t2 = relu(127 - t1)               = clamp(u/2, 0, 127)
            # t3 = t2 + 1536 (f16 write rounds), t4 = t3 - 1536
            nc.scalar.activation(act[:], act[:], Act.Relu,
                                 bias=biases[127.0 + 63.5 * ksum][:],
                                 scale=-0.5)
            nc.scalar.activation(act[:], act[:], Act.Relu,
                                 bias=biases[127.0][:], scale=-1.0)
            nc.scalar.activation(act[:], act[:], Act.Identity,
                                 bias=biases[OFF][:], scale=1.0)
            nc.scalar.activation(act[:], act[:], Act.Identity,
                                 bias=biases[-OFF][:], scale=1.0)
            for h in range(2):
                pT = pst.tile([128, 1024], f16, name="pT")
                for j in range(4):
                    a = h * 4 + j
                    nc.tensor.transpose(pT[:, j * 128:(j + 1) * 128],
                                        act[:, a * 144:a * 144 + 128],
                                        ident[:])
                    nc.tensor.transpose(
                        pT[0:16, 512 + j * 128:512 + (j + 1) * 128],
                        act[:, a * 144 + 128:(a + 1) * 144], ident[:])
                aT = tpool.tile([128, 1024], f16, name="aT")
                nc.scalar.copy(aT[:], pT[:])
                pOT = po.tile([10, 512], f32, name="pOT")
                nc.tensor.matmul(pOT[:], w1t[:], aT[:, 0:512], start=True,
                                 stop=False)
                nc.tensor.matmul(pOT[:], w2t[:], aT[0:16, 512:1024],
                                 start=False, stop=True)
                soT = spool.tile([10, 512], f32, name="soT")
                nc.scalar.copy(soT[:], pOT[:])
                nc.sync.dma_start(
                    out[:, g * 1024 + h * 512:g * 1024 + (h + 1) * 512],
                    soT[:])

        # software pipeline: stage A of group g overlaps stage B of g-1 and
        # stage C of g-2, so no engine's FIFO interleaves early and late
        # stages of the same group
        stA, stB = {}, {}
        for g in range(NGROUPS + 2):
            if g < NGROUPS:
                stA[g] = stage_a(g)
            if 1 <= g < NGROUPS + 1:
                stB[g - 1] = stage_b(g - 1, stA.pop(g - 1))
            if g >= 2:
                stage_c(g - 2, stB.pop(g - 2))

    nc.compile()
    return nc


def _prep(conv_w, fc_w):
    # replicate reference weight quantization exactly (all steps exact in f32)
    cw = np.asarray(conv_w, np.float32).reshape(3, 3)
    sgn9 = tuple(int(v) for v in
                 np.round(np.clip(cw, -0.5, 0.5) * 2.0).flatten())
    fw = np.asarray(fc_w, np.float32)
    wfq = (np.round(np.clip(fw, -0.5, 0.5) * 2.0) / 2.0 / 8.0).astype(
        np.float32)
    # FC sees act/128; fold the /128 into W (values m/2048, exact fp16).
    # Feature k = i*12 + j of the 12x12 pool interior maps to fc column
    # (i+1)*14 + (j+1) of the 14x14 padded pool grid.
    Wdev = np.zeros((144, 10), np.float32)
    for i in range(12):
        for j in range(12):
            Wdev[i * 12 + j, :] = wfq[:, (i + 1) * 14 + (j + 1)] / 128.0
    W1 = np.ascontiguousarray(Wdev[0:128]).astype(np.float16)
    W2 = np.ascontiguousarray(Wdev[128:144]).astype(np.float16)
    return sgn9, W1, W2


def _get_program(sgn9):
    nc = _cache.get(sgn9)
    if nc is None:
        nc = _build(sgn9)
        _cache[sgn9] = nc
    return nc


def _make_in_maps(x2d, W1, W2):
    return [{"x": np.ascontiguousarray(x2d[c * NPC:(c + 1) * NPC]),
             "w1": W1, "w2": W2} for c in range(NCORES)]


def run(x, conv_w, fc_w, trace=False, **kw):
    from concourse.bass_utils import run_bass_kernel_spmd

    x2d = np.ascontiguousarray(
        np.asarray(x, np.float32).reshape(B, 576))
    sgn9, W1, W2 = _prep(conv_w, fc_w)
    nc = _get_program(sgn9)
    res = run_bass_kernel_spmd(nc, _make_in_maps(x2d, W1, W2),
                               core_ids=list(range(NCORES)),
                               trace=trace, **kw)
    out = np.concatenate([np.asarray(r["out"]).T for r in res.results],
                         axis=0)
    return np.ascontiguousarray(out.astype(np.float32)), res


def kernel(x, conv_w, fc_w):
    out, _ = run(x, conv_w, fc_w, trace=False)
    return out


# revision 16
# speedup vs baseline: 4.7392x; 1.0006x over previous
import sys

if "/opt/trn_rl_repo" not in sys.path:
    sys.path.insert(0, "/opt/trn_rl_repo")

import numpy as np

NCORES = 8
B = 65536
NPC = B // NCORES  # 8192 images per core
G = 8              # image-tiles (of 128) per group
NGROUPS = NPC // (128 * G)
AF = 128.0 / 127.5
OFF = 1536.0   # f16 magic: +1536 makes the f16 write round |v|<512 to int
PAD = 127.0    # padded-cell value == quantized level of A=0, post-clamp space
# padded image slot: lead spacer row of 26, then per image 25 rows x 26 cols
# (24x24 data at rows 0..23, cols 1..24; row 24 is the inter-image spacer)
SLOT = 650     # 25 * 26
L = 26 + G * SLOT  # 5226 padded elements per partition

_cache = {}


def _build(sgn9):
    """sgn9: tuple of 9 ints in {-1,0,1}: conv taps (2x quantized), row-major.

    Layout: partition = image, free = padded pixels. All 3x3 taps become
    single contiguous shifted adds over the full tile; padded cells hold
    127 (== A=0 in the offset space xh = clamp(A,-127,127)+127) so every
    data pixel receives every tap and the uniform offset 127*sum(signs)
    is folded into the activation-quant bias. Partial sums stay < 2048 in
    magnitude for <= 8 accumulation terms, so f16 accumulation is exact.
    """
    from contextlib import ExitStack

    import concourse.tile as tile
    import concourse.masks as masks
    from concourse import bacc, mybir

    f32 = mybir.dt.float32
    f16 = mybir.dt.float16
    Alu = mybir.AluOpType
    Act = mybir.ActivationFunctionType

    nc = bacc.Bacc("TRN2", target_bir_lowering=False, debug=False,
                   num_devices=NCORES)
    x = nc.dram_tensor("x", [NPC, 576], f32, kind="ExternalInput").ap()
    w1 = nc.dram_tensor("w1", [128, 10], f16, kind="ExternalInput").ap()
    w2 = nc.dram_tensor("w2", [16, 10], f16, kind="ExternalInput").ap()
    out = nc.dram_tensor("out", [10, NPC], f32, kind="ExternalOutput").ap()

    cen = float(sgn9[4])
    taps = [(26 * dr + dc, float(sgn9[(dr + 1) * 3 + (dc + 1)]))
            for dr in (-1, 0, 1) for dc in (-1, 0, 1)
            if not (dr == 0 and dc == 0)
            and sgn9[(dr + 1) * 3 + (dc + 1)] != 0]
    ksum = cen + sum(s for _, s in taps)  # offset multiplier (uniform)
    # GpSimd initializes ya with a plain shifted copy of a positive tap
    # (only possible when there is no center contribution)
    ci = next((i for i, (_, s) in enumerate(taps) if s > 0), None)
    if cen == 0.0 and ci is not None:
        copy_tap = taps.pop(ci)
    else:
        copy_tap = None
    # GpSimd also takes one accumulate tap (tt add/sub is supported there)
    p_taps, v_taps = (taps[:1], taps[1:]) if len(taps) > 4 else ([], taps)
    # 5-op factored plan (P/M pair streams) for this specific tap pattern
    FACTORED = sgn9 == (1, -1, -1, 1, 0, 1, -1, 0, 1)

    with tile.TileContext(nc) as tc, ExitStack() as ctx:
        consts = ctx.enter_context(tc.tile_pool(name="consts", bufs=1))
        w1t = consts.tile([128, 10], f16)
        w2t = consts.tile([16, 10], f16)
        ident = consts.tile([128, 128], f16)
        nc.sync.dma_start(w1t[:], w1)
        nc.sync.dma_start(w2t[:], w2)
        masks.make_identity(nc, ident[:])
        # per-partition f32 bias constants for non-Copy activations
        bias_vals = [OFF + 127.0, 254.0, 127.0 + 63.5 * ksum, 127.0,
                     OFF, -OFF]
        biases = {}
        for i, v in enumerate(bias_vals):
            bt = consts.tile([128, 1], f32, name=f"bias{i}")
            nc.gpsimd.memset(bt[:], v)
            biases[v] = bt

        xpool = ctx.enter_context(tc.tile_pool(name="xp", bufs=2))
        hpool = ctx.enter_context(tc.tile_pool(name="hp", bufs=1))
        spool_s = ctx.enter_context(tc.tile_pool(name="xs", bufs=2))
        yapool = ctx.enter_context(tc.tile_pool(name="yap", bufs=2))
        ptpool = ctx.enter_context(tc.tile_pool(name="ptp", bufs=2))
        mtpool = ctx.enter_context(tc.tile_pool(name="mtp", bufs=2))
        ppool = ctx.enter_context(tc.tile_pool(name="pp", bufs=2))
        apool = ctx.enter_context(tc.tile_pool(name="ap", bufs=2))
        tpool = ctx.enter_context(tc.tile_pool(name="tp", bufs=4))
        spool = ctx.enter_context(tc.tile_pool(name="sp", bufs=2))
        pst = ctx.enter_context(tc.tile_pool(name="pst", bufs=2, space="PSUM"))
        po = ctx.enter_context(tc.tile_pool(name="po", bufs=2, space="PSUM"))

        # manual double-buffer for xh: padded cells are initialized once and
        # never rewritten (quant + clamps touch only data cells)
        xhs = [hpool.tile([128, L], f16, name=f"xh{i}") for i in range(2)]
        for xh in xhs:
            xv = xh[:, 26:].rearrange("p (a r c) -> p a r c", r=25, c=26)
            nc.gpsimd.memset(xh[:, 0:26], PAD)         # lead spacer row
            nc.gpsimd.memset(xv[:, :, 24, :], PAD)     # inter-image spacers
            nc.gpsimd.memset(xv[:, :, 0:24, 0:1], PAD)   # left border col
            nc.gpsimd.memset(xv[:, :, 0:24, 25:26], PAD)  # right border col

        xv_dram = x.rearrange("(g a p) f -> g p a f", p=128, a=G)

        def stage_a(g):
            """DMA in, quantize + clamp (Scalar), start conv (GpSimd)."""
            xt = xpool.tile([128, G * 576], f32, name="xt")
            nc.sync.dma_start(xt[:].rearrange("p (a f) -> p a f", a=G),
                              xv_dram[g])
            xh = xhs[g % 2]
            xv = xh[:, 26:].rearrange("p (a r c) -> p a r c", r=25, c=26)
            xdv = xv[:, :, 0:24, 1:25]
            # quant: xh' = A + 1536 via f16 magic write (A = round(x*AF-128))
            nc.scalar.activation(
                xdv, xt[:].rearrange("p (a r c) -> p a r c", r=24, c=24),
                Act.Copy, bias=OFF - 128.0, scale=AF)
            # both clamps via two Relus (scalar engine), leaving
            # xh = clamp(A, -127, 127) + 127 in [0, 254]
            xs = spool_s.tile([128, G * 576], f16, name="xs")
            xsv = xs[:].rearrange("p (a r c) -> p a r c", r=24, c=24)
            nc.scalar.activation(xsv, xdv, Act.Relu,
                                 bias=biases[OFF + 127.0][:], scale=-1.0)
            nc.scalar.activation(xdv, xsv, Act.Relu,
                                 bias=biases[254.0][:], scale=-1.0)

            ya = yapool.tile([128, L], f16, name="ya")
            yv = ya[:, 26:].rearrange("p (a r c) -> p a r c", r=25, c=26)
            if FACTORED:
                # P = x(c-1) + x(c+1) on GpSimd, overlaps freely (no chain)
                pt = ptpool.tile([128, L], f16, name="pt")
                nc.gpsimd.tensor_tensor(pt[:, 1:L - 1], xh[:, 0:L - 2],
                                        xh[:, 2:L], Alu.add)
                return xh, ya, pt
            # generic path: init ya + GpSimd accumulate taps
            if copy_tap is not None:
                s0, _ = copy_tap
                b0, b1 = max(0, -s0), L - max(0, s0)
                if b0 > 0:
                    nc.gpsimd.memset(ya[:, 0:b0], 0.0)
                if b1 < L:
                    nc.gpsimd.memset(ya[:, b1:L], 0.0)
                nc.gpsimd.tensor_copy(ya[:, b0:b1], xh[:, b0 + s0:b1 + s0])
            elif cen != 0.0:
                nc.scalar.activation(yv[:, :, 0:24, 1:25], xdv, Act.Copy,
                                     bias=0.0, scale=cen)
            else:
                nc.gpsimd.memset(ya[:], 0.0)
            for off, s in p_taps:
                b0, b1 = max(0, -off), L - max(0, off)
                nc.gpsimd.tensor_tensor(
                    ya[:, b0:b1], ya[:, b0:b1], xh[:, b0 + off:b1 + off],
                    Alu.add if s > 0 else Alu.subtract)
            return xh, ya, None

        def stage_b(g, st):
            """DVE conv + maxpool."""
            xh, ya, pt = st
            yv = ya[:, 26:].rearrange("p (a r c) -> p a r c", r=25, c=26)
            if FACTORED:
                # rows of sgn9 are r0=(1,-1,-1), r1=(1,0,1), r2=(-1,0,1):
                # with P = x(c-1)+x(c+1), M = x(c+1)-x(c-1):
                #   y(r) = P(r) + M(r+1) - M(r-1) - x(r-1)
                mt = mtpool.tile([128, L], f16, name="mt")
                nc.vector.tensor_tensor(mt[:, 1:L - 1], xh[:, 2:L],
                                        xh[:, 0:L - 2], Alu.subtract)
                nc.vector.tensor_tensor(ya[:, 1:L - 27], pt[:, 1:L - 27],
                                        mt[:, 27:L - 1], Alu.add)
                nc.vector.tensor_tensor(ya[:, 27:L - 1], ya[:, 27:L - 1],
                                        mt[:, 1:L - 27], Alu.subtract)
                nc.vector.tensor_tensor(ya[:, 26:L], ya[:, 26:L],
                                        xh[:, 0:L - 26], Alu.subtract)
            else:
                for off, s in v_taps:
                    b0, b1 = max(0, -off), L - max(0, off)
                    nc.vector.tensor_tensor(
                        ya[:, b0:b1], ya[:, b0:b1], xh[:, b0 + off:b1 + off],
                        Alu.add if s > 0 else Alu.subtract)
            # maxpool 2x2: column pairs first (stride-2 reads, 1x mode),
            # then row pairs on the half-sized tensor (packed, 2x mode)
            yu = yv[:, :, 0:24, 1:25].rearrange("p a r (c u) -> p a r c u",
                                                u=2)
            p1 = ppool.tile([128, G * 288], f16, name="p1")
            p1v = p1[:].rearrange("p (a r c) -> p a r c", r=24, c=12)
            nc.vector.tensor_tensor(p1v, yu[:, :, :, :, 0],
                                    yu[:, :, :, :, 1], Alu.max)
            act = apool.tile([128, G * 144], f16, name="act")
            p1w = p1[:].rearrange("p (a r t c) -> p a r t c",
                                  r=12, t=2, c=12)
            actr = act[:].rearrange("p (a r c) -> p a r c", r=12, c=12)
            nc.vector.tensor_tensor(actr, p1w[:, :, :, 0, :],
                                    p1w[:, :, :, 1, :], Alu.max)
            return act

        def stage_c(g, act):
            """Act-quant (Scalar), PE transpose + FC, output DMA."""
            #   u = p2 - 127*ksum;  act = round(clamp(u/2, 0, 127))
            # t1 = relu(127 + 63.5*ksum - p2/2) = 127 - min(u/2, 127)
            # t2 = relu(127 - t1)               = clamp(u/2, 0, 127)
            # t3 = t2 + 1536 (f16 write rounds), t4 = t3 - 1536
            nc.scalar.activation(act[:], act[:], Act.Relu,
                                 bias=biases[127.0 + 63.5 * ksum][:],
                                 scale=-0.5)
            nc.scalar.activation(act[:], act[:], Act.Relu,
                                 bias=biases[127.0][:], scale=-1.0)
            nc.scalar.activation(act[:], act[:], Act.Identity,
                                 bias=biases[OFF][:], scale=1.0)
            nc.scalar.activation(act[:], act[:], Act.Identity,
                                 bias=biases[-OFF][:], scale=1.0)
            for h in range(2):
                pT = pst.tile([128, 1024], f16, name="pT")
                for j in range(4):
                    a = h * 4 + j
                    nc.tensor.transpose(pT[:, j * 128:(j + 1) * 128],
                                        act[:, a * 144:a * 144 + 128],
                                        ident[:])
                    nc.tensor.transpose(
                        pT[0:16, 512 + j * 128:512 + (j + 1) * 128],
                        act[:, a * 144 + 128:(a + 1) * 144], ident[:])
                aT = tpool.tile([128, 1024], f16, name="aT")
                nc.scalar.copy(aT[:], pT[:])
                pOT = po.tile([10, 512], f32, name="pOT")
                nc.tensor.matmul(pOT[:], w1t[:], aT[:, 0:512], start=True,
                                 stop=False)
                nc.tensor.matmul(pOT[:], w2t[:], aT[0:16, 512:1024],
                                 start=False, stop=True)
                soT = spool.tile([10, 512], f32, name="soT")
                nc.scalar.copy(soT[:], pOT[:])
                nc.sync.dma_start(
                    out[:, g * 1024 + h * 512:g * 1024 + (h + 1) * 512],
                    soT[:])

        # software pipeline: stage A of group g overlaps stage B of g-1 and
        # stage C of g-2, so no engine's FIFO interleaves early and late
        # stages of the same group
        stA, stB = {}, {}
        for g in range(NGROUPS + 2):
            if g < NGROUPS:
                stA[g] = stage_a(g)
            if 1 <= g < NGROUPS + 1:
                stB[g - 1] = stage_b(g - 1, stA.pop(g - 1))
            if g >= 2:
                stage_c(g - 2, stB.pop(g - 2))

    nc.compile()
    return nc


def _prep(conv_w, fc_w):
    # replicate reference weight quantization exactly (all steps exact in f32)
    cw = np.asarray(conv_w, np.float32).reshape(3, 3)
    sgn9 = tuple(int(v) for v in
                 np.round(np.clip(cw, -0.5, 0.5) * 2.0).flatten())
    fw = np.asarray(fc_w, np.float32)
    wfq = (np.round(np.clip(fw, -0.5, 0.5) * 2.0) / 2.0 / 8.0).astype(
        np.float32)
    # FC sees act/128; fold the /128 into W (values m/2048, exact fp16).
    # Feature k = i*12 + j of the 12x12 pool interior maps to fc column
    # (i+1)*14 + (j+1) of the 14x14 padded pool grid.
    Wdev = np.zeros((144, 10), np.float32)
    for i in range(12):
        for j in range(12):
            Wdev[i * 12 + j, :] = wfq[:, (i + 1) * 14 + (j + 1)] / 128.0
    W1 = np.ascontiguousarray(Wdev[0:128]).astype(np.float16)
    W2 = np.ascontiguousarray(Wdev[128:144]).astype(np.float16)
    return sgn9, W1, W2


def _get_program(sgn9):
    nc = _cache.get(sgn9)
    if nc is None:
        nc = _build(sgn9)
        _cache[sgn9] = nc
    return nc


def _make_in_maps(x2d, W1, W2):
    return [{"x": np.ascontiguousarray(x2d[c * NPC:(c + 1) * NPC]),
             "w1": W1, "w2": W2} for c in range(NCORES)]


def run(x, conv_w, fc_w, trace=False, **kw):
    from concourse.bass_utils import run_bass_kernel_spmd

    x2d = np.ascontiguousarray(
        np.asarray(x, np.float32).reshape(B, 576))
    sgn9, W1, W2 = _prep(conv_w, fc_w)
    nc = _get_program(sgn9)
    res = run_bass_kernel_spmd(nc, _make_in_maps(x2d, W1, W2),
                               core_ids=list(range(NCORES)),
                               trace=trace, **kw)
    out = np.concatenate([np.asarray(r["out"]).T for r in res.results],
                         axis=0)
    return np.ascontiguousarray(out.astype(np.float32)), res


def kernel(x, conv_w, fc_w):
    out, _ = run(x, conv_w, fc_w, trace=False)
    return out
